# revision 45
# baseline (speedup 1.0000x reference)
"""Self-contained Trainium2 Bass kernel: GQA multi-head attention.

Problem (nn_MultiHeadAttention): B=1, S=4096, H=2048, 16 query heads,
4 KV heads (GQA groups of 4), additive attention mask, fp32 reference:

    q = (x @ Wq), k = (x @ Wk), v = (x @ Wv)       # per-head HD=128
    scores = q k^T / sqrt(HD) + mask ; p = softmax(scores)
    out = (p v) concat-heads @ Wo

Sharding: tensor-parallel over heads across 8 NeuronCores — each core owns
2 query heads + their 1 shared KV head and the matching 256-row slice of
Wo. Each core computes its partial output projection (S, H); the host sums
the 8 partials (the TP all-reduce) and returns the full output.

On-core dataflow (per core, all matmuls bf16 with fp32 PSUM accumulate):
  phase 1: project Q^T/K^T (head-dim on partitions) and V (seq on
           partitions) from X^T slabs streamed through SBUF.
  phase 2: per 512-wide query window, scores^T tiles (k on partitions) =
           K_chunk @ Q^T, exp on ScalarE (scale folded in), causal handled
           by skipping above-diagonal chunks + one triangular 0/1 multiply
           on the diagonal chunk; attn@V via PE with a fused ones-column in
           V so the softmax denominators fall out of the same matmuls;
           normalize with per-partition reciprocal.
  phase 3: PE-transpose of the per-head attention output, then the partial
           output projection, DMA'd to DRAM in fp32.
"""

import os

import numpy as np
import ml_dtypes

BF16 = ml_dtypes.bfloat16
S_FULL, H, HD = 4096, 2048, 128
NCORES = 8
QW = 512  # query window (psum bank width in fp32)
KC = 128  # key chunk (contraction tile for attn@V)

_compiled = {}
LAST_RESULT = None


def _build(variant: str, S: int = S_FULL, num_devices: int = NCORES, dbg: bool = False):
    import concourse.mybir as mybir
    import concourse.tile as tile
    from concourse import bacc
    from concourse.masks import make_identity, make_upper_triangular

    fp32 = mybir.dt.float32
    bf16 = mybir.dt.bfloat16
    Act = mybir.ActivationFunctionType

    KO = H // 128            # hidden-dim contraction chunks
    NWIN = S // QW           # query windows
    NCH = S // KC            # key chunks
    QTR = 512 if S % 512 == 0 else S  # phase-1 sequence slab (one query window)
    NQTR = S // QTR
    WPQ = QTR // QW
    VPQ = QTR // KC
    NT = S // 128            # 128-row tiles along S
    SCALE = 1.0 / float(np.sqrt(HD))
    causal = variant == "causal"

    nc = bacc.Bacc(
        "TRN2", target_bir_lowering=False, debug=False, num_devices=num_devices
    )
    xt_d = nc.dram_tensor("xt", (H, S), bf16, kind="ExternalInput").ap()
    wq_d = nc.dram_tensor("wq", (H, 256), bf16, kind="ExternalInput").ap()
    wk_d = nc.dram_tensor("wk", (H, 128), bf16, kind="ExternalInput").ap()
    wv_d = nc.dram_tensor("wv", (H, 128), bf16, kind="ExternalInput").ap()
    wo_d = nc.dram_tensor("wo", (256, H), bf16, kind="ExternalInput").ap()
    if variant == "dense_mask":
        mt_d = nc.dram_tensor("maskt", (S, S), bf16, kind="ExternalInput").ap()
    out_d = nc.dram_tensor("out", (S, H), fp32, kind="ExternalOutput").ap()
    if dbg:
        NCH_ = S // KC
        dbg_q = nc.dram_tensor("dbg_q", (128, S), mybir.dt.bfloat16, kind="ExternalOutput").ap()
        dbg_k = nc.dram_tensor("dbg_k", (128, S), mybir.dt.bfloat16, kind="ExternalOutput").ap()
        dbg_v = nc.dram_tensor("dbg_v", (128, NCH_, 132), mybir.dt.bfloat16, kind="ExternalOutput").ap()
        dbg_a = nc.dram_tensor("dbg_a", (128, S // 128, 128), mybir.dt.bfloat16, kind="ExternalOutput").ap()
        dbg_at = nc.dram_tensor("dbg_at", (128, S), mybir.dt.bfloat16, kind="ExternalOutput").ap()
        dbg_p = nc.dram_tensor("dbg_p", (128, QW), mybir.dt.bfloat16, kind="ExternalOutput").ap()

    with tile.TileContext(nc) as tc:
        with tc.tile_pool(name="persist", bufs=1) as persist:
            ident = persist.tile([128, 128], bf16, tag="ident", name="ident")
            if causal:
                tri = persist.tile([128, 128], bf16, tag="tri", name="tri")
            q_sb = [
                persist.tile([128, S], bf16, tag=f"qsb{h}", name=f"qsb{h}")
                for h in range(2)
            ]
            k_sb = persist.tile([128, S], bf16, tag="ksb", name="ksb")
            v_sb = persist.tile([128, NCH, 132], bf16, tag="vsb", name="vsb")
            a_nm = [
                persist.tile([128, NT, 128], bf16, tag=f"anm{h}", name=f"anm{h}")
                for h in range(2)
            ]
            at_sb = [
                persist.tile([128, S], bf16, tag=f"atsb{h}", name=f"atsb{h}")
                for h in range(2)
            ]
            wo_sb = persist.tile([128, 2, H], bf16, tag="wosb", name="wosb")

            nc.vector.memset(v_sb[:, :, 128:129], 1.0)

            ncopy = [0]

            def copy_any(dst, src):
                # alternate ScalarE / VectorE so PSUM-evacuation copies don't
                # pile onto one engine
                if ncopy[0] % 2 == 0:
                    nc.scalar.copy(dst, src)
                else:
                    nc.vector.tensor_copy(dst, src)
                ncopy[0] += 1

            # ---------------- phase 1: projections ----------------
            with tc.tile_pool(name="wts", bufs=1) as wts, tc.tile_pool(
                name="xtp", bufs=3
            ) as xtp, tc.tile_pool(name="vtp", bufs=2) as vtp, tc.tile_pool(
                name="ps1", bufs=3, space="PSUM"
            ) as ps1:
                wq_sb = wts.tile([128, KO, 256], bf16, tag="wq", name="wq_sb")
                wk_sb = wts.tile([128, KO, 128], bf16, tag="wk", name="wk_sb")
                wv_sb = wts.tile([128, KO, 128], bf16, tag="wv", name="wv_sb")
                nc.sync.dma_start(wk_sb[:], wk_d.rearrange("(o p) n -> p o n", p=128))
                nc.gpsimd.dma_start(wv_sb[:], wv_d.rearrange("(o p) n -> p o n", p=128))
                nc.gpsimd.dma_start(wq_sb[:], wq_d.rearrange("(o p) n -> p o n", p=128))
                xt_r = xt_d.rearrange("(o p) s -> p o s", p=128)
                for qi in range(NQTR):
                    s0 = qi * QTR
                    xt_t = xtp.tile([128, KO, QTR], bf16, tag="xt", name=f"xt{qi}")
                    for o in range(KO):
                        # first slabs also borrow the idle ScalarE HWDGE queue;
                        # later slabs keep ScalarE free for exp
                        if qi < 2:
                            eng = (nc.sync, nc.gpsimd, nc.scalar)[o % 3]
                        else:
                            eng = nc.sync if o % 2 == 0 else nc.gpsimd
                        eng.dma_start(xt_t[:, o, :], xt_r[:, o, s0 : s0 + QTR])
                    if qi == 0:
                        # mask generation on gpsimd queues AFTER the first
                        # slab's DMA issues so they don't delay the pipeline
                        make_identity(nc, ident)
                        if causal:
                            make_upper_triangular(nc, tri, val=1.0, diag=True)
                    for wi in range(WPQ):
                        w = qi * WPQ + wi
                        kp = ps1.tile([128, QW], fp32, tag="kp", name=f"kp{w}")
                        for o in range(KO):
                            nc.tensor.matmul(
                                kp,
                                lhsT=wk_sb[:, o, :],
                                rhs=xt_t[:, o, wi * QW : (wi + 1) * QW],
                                start=(o == 0),
                                stop=(o == KO - 1),
                            )
                        copy_any(k_sb[:, w * QW : (w + 1) * QW], kp)
                    for h in range(2):
                        for wi in range(WPQ):
                            w = qi * WPQ + wi
                            qp = ps1.tile([128, QW], fp32, tag="kp", name=f"qp{h}_{w}")
                            for o in range(KO):
                                nc.tensor.matmul(
                                    qp,
                                    lhsT=wq_sb[:, o, h * 128 : (h + 1) * 128],
                                    rhs=xt_t[:, o, wi * QW : (wi + 1) * QW],
                                    start=(o == 0),
                                    stop=(o == KO - 1),
                                )
                            copy_any(q_sb[h][:, w * QW : (w + 1) * QW], qp)
                    for vi in range(VPQ):
                        st = qi * VPQ + vi
                        vp = ps1.tile([128, 128], fp32, tag="vp", name=f"vp{st}")
                        for o in range(KO):
                            nc.tensor.matmul(
                                vp,
                                lhsT=xt_t[:, o, vi * 128 : (vi + 1) * 128],
                                rhs=wv_sb[:, o, :],
                                start=(o == 0),
                                stop=(o == KO - 1),
                            )
                        copy_any(v_sb[:, st, :128], vp)

            # ------- phase 2+3 fused: attention + output projection, streamed
            # per query window so the output-projection work overlaps the
            # attention of later windows instead of forming a serial tail.
            with tc.tile_pool(name="probs", bufs=4) as probs_p, tc.tile_pool(
                name="ps_s", bufs=2, space="PSUM"
            ) as ps_s, tc.tile_pool(
                name="ps_av", bufs=1, space="PSUM"
            ) as ps_av, tc.tile_pool(name="rcp", bufs=4) as rcp_p, tc.tile_pool(
                name="mtp", bufs=4
            ) as mtp, tc.tile_pool(
                name="ps_o", bufs=2, space="PSUM"
            ) as ps_o, tc.tile_pool(name="outp", bufs=4) as outp:
                # Wo is first needed at window 0's output projection — load it
                # off the startup critical path
                nc.gpsimd.dma_start(wo_sb[:], wo_d.rearrange("(h p) n -> p h n", p=128))
                nout = [0]

                # Output-side work (transpose + O-proj) for window w is emitted
                # interleaved between window w+1's score/AV pairs: the PE stream
                # is static, so ready O-proj matmuls must sit between AV groups
                # to fill the slots where AV waits on the exp.
                def make_tr_thunk(h, t, use_act):
                    def f(pool=None, tag=None):
                        trp = (pool or ps_o).tile(
                            [128, 128], bf16, tag=tag or "po", name=f"tr{h}_{t}"
                        )
                        nc.tensor.transpose(trp, a_nm[h][:, t, :], ident)
                        if use_act:
                            nc.scalar.copy(at_sb[h][:, t * 128 : (t + 1) * 128], trp)
                        else:
                            nc.vector.tensor_copy(
                                at_sb[h][:, t * 128 : (t + 1) * 128], trp
                            )
                    return f

                def make_po_thunk(qt, ns, use_act):
                    def f(pool=None, tag=None):
                        po = (pool or ps_o).tile(
                            [128, 512], fp32, tag=tag or "po", name=f"po{qt}_{ns}"
                        )
                        for h in range(2):
                            nc.tensor.matmul(
                                po,
                                lhsT=at_sb[h][:, qt * 128 : (qt + 1) * 128],
                                rhs=wo_sb[:, h, ns * 512 : (ns + 1) * 512],
                                start=(h == 0),
                                stop=(h == 1),
                            )
                        ob = outp.tile([128, 512], fp32, tag="ob", name=f"ob{qt}_{ns}")
                        if use_act:
                            nc.scalar.copy(ob, po)
                        else:
                            nc.vector.tensor_copy(ob, po)
                        nout[0] += 1
                        eng = nc.sync if nout[0] % 2 == 0 else nc.gpsimd
                        eng.dma_start(
                            out_d[qt * 128 : (qt + 1) * 128, ns * 512 : (ns + 1) * 512],
                            ob,
                        )
                    return f

                def out_thunks(w, act_share):
                    # per query tile: both transposes then its 4 O-proj slices,
                    # so the 2 shared PSUM slots chain tightly. act_share puts
                    # every other copy on ScalarE (use when ACT has exp slack).
                    th = []
                    i = 0
                    for qt in range(4 * w, 4 * w + 4):
                        for h in range(2):
                            th.append(make_tr_thunk(h, qt, act_share and i % 2 == 0))
                            i += 1
                        for ns in range(H // 512):
                            th.append(make_po_thunk(qt, ns, act_share and i % 2 == 0))
                            i += 1
                    return th

                pending = []
                for w in range(NWIN):
                    q0 = w * QW
                    k_done = [0]
                    pairs_total = 2 * (((q0 // KC if causal else NCH) + (4 if causal else 0)) // 2)
                    pairs_done = [0]

                    def drain_pending():
                        want = (pairs_done[0] * len(pending)) // max(pairs_total, 1)
                        while k_done[0] < want:
                            pending[k_done[0]]()
                            k_done[0] += 1

                    for h in range(2):
                        n_full = (q0 // KC) if causal else NCH
                        n_ch = (n_full + 4) if causal else NCH
                        # av packs 2 q-subs per PSUM bank. start=True clears the
                        # whole bank, so only subs 0 and 2 (first in their bank)
                        # use start=True; subs 1 and 3 rely on "overwrite where
                        # has_written is clear" for their first accumulation.
                        av = ps_av.tile(
                            [128, 4, 256], fp32, tag="av", name=f"av{h}_{w}"
                        )
                        # cover the first scores->exp latency of this head with
                        # a couple of ready output-projection units
                        for _ in range(2):
                            if k_done[0] < len(pending):
                                pending[k_done[0]]()
                                k_done[0] += 1
                        # chunks processed in pairs sharing one [128,2,512]
                        # psum tile so full pairs need only ONE 1024-wide exp
                        for p in range(n_ch // 2):
                            cpair = (2 * p, 2 * p + 1)
                            sc = ps_s.tile(
                                [128, 2, QW], fp32, tag="sc", name=f"sc{h}_{w}_{p}"
                            )
                            pt = probs_p.tile(
                                [128, 2, QW], bf16, tag="pt", name=f"pt{h}_{w}_{p}"
                            )
                            halves = []
                            for j, c in enumerate(cpair):
                                d = c - n_full  # >= 0 -> diagonal chunk (causal)
                                col0 = d * 128 if (causal and d >= 0) else 0
                                halves.append((j, c, d, col0))
                                nc.tensor.matmul(
                                    sc[:, j, col0:],
                                    lhsT=k_sb[:, c * KC : (c + 1) * KC],
                                    rhs=q_sb[h][:, q0 + col0 : q0 + QW],
                                    start=True,
                                    stop=True,
                                )
                                if variant == "dense_mask":
                                    mt_t = mtp.tile(
                                        [128, QW], bf16, tag="mt", name=f"mt{h}_{w}_{c}"
                                    )
                                    nc.sync.dma_start(
                                        mt_t[:], mt_d[c * KC : (c + 1) * KC, q0 : q0 + QW]
                                    )
                                    nc.vector.tensor_add(sc[:, j, :], sc[:, j, :], mt_t)
                            if causal and cpair[0] >= n_full:
                                # diagonal pair: per-half exp on the valid region
                                for j, c, d, col0 in halves:
                                    nc.scalar.activation(
                                        pt[:, j, col0:], sc[:, j, col0:], Act.Exp, scale=SCALE
                                    )
                                    nc.vector.tensor_mul(
                                        pt[:, j, col0 : col0 + 128],
                                        pt[:, j, col0 : col0 + 128],
                                        tri,
                                    )
                            elif w < 3:
                                # short windows are exp-LATENCY bound: per-half
                                # exp lets AV of half 0 start ~600ns earlier
                                for j in (0, 1):
                                    nc.scalar.activation(
                                        pt[:, j, :], sc[:, j, :], Act.Exp, scale=SCALE
                                    )
                            else:
                                nc.scalar.activation(pt[:], sc[:], Act.Exp, scale=SCALE)
                            if dbg and h == 0 and w == 0 and p == 0:
                                nc.sync.dma_start(dbg_p[:], pt[:, 0, :])
                            for j, c, d, col0 in halves:
                                s_lo = d if (causal and d >= 0) else 0
                                for s in range(s_lo, 4):
                                    last_c = (n_full + s) if causal else (NCH - 1)
                                    nc.tensor.matmul(
                                        av[:, s, :129],
                                        lhsT=pt[:, j, s * 128 : (s + 1) * 128],
                                        rhs=v_sb[:, c, :129],
                                        start=(c == 0 and s % 2 == 0),
                                        stop=(c == last_c),
                                    )
                            pairs_done[0] += 1
                            drain_pending()
                        # evacuate av psum with one copy (fast bank release),
                        # then normalize from SBUF on DVE
                        avc = rcp_p.tile([128, 4, 132], fp32, tag="avc", name=f"avc{h}_{w}")
                        nc.vector.tensor_copy(avc[:], av[:, :, :132])
                        rc = rcp_p.tile([128, 4], fp32, tag="rc", name=f"rc{h}_{w}")
                        for s in range(4):
                            nc.vector.reciprocal(rc[:, s : s + 1], avc[:, s, 128:129])
                            nc.vector.tensor_scalar_mul(
                                a_nm[h][:, w * 4 + s, :],
                                avc[:, s, :128],
                                rc[:, s : s + 1],
                            )
                    # flush any remainder of the previous window's output work,
                    # then queue this window's for interleaved emission
                    while k_done[0] < len(pending):
                        pending[k_done[0]]()
                        k_done[0] += 1
                    if w < NWIN - 1:
                        # ACT is exp-saturated through the whole attention
                        # phase — keep all interleaved copies on DVE
                        pending = out_thunks(w, act_share=False)
                # tail: the final window's output work has nothing to interleave
                # with — ACT is exp-free (alternate copy engines) and the
                # attention PSUM pools are idle (rotate tiles across their
                # slots so the transpose->O-proj chains run wide)
                tail_pools = [(None, None), (ps_s, "sc"), (ps_av, "av")]
                for i, f in enumerate(out_thunks(NWIN - 1, act_share=True)):
                    pool, tag = tail_pools[i % 3]
                    f(pool, tag)

            if dbg:
                nc.sync.dma_start(dbg_q[:], q_sb[0][:])
                nc.sync.dma_start(dbg_k[:], k_sb[:])
                nc.sync.dma_start(dbg_v[:], v_sb[:])
                nc.sync.dma_start(dbg_a[:], a_nm[0][:])
                nc.sync.dma_start(dbg_at[:], at_sb[0][:])

    nc.compile()
    return nc


def _get_compiled(variant: str, S: int = S_FULL, num_devices: int = NCORES):
    key = (variant, S, num_devices)
    if key not in _compiled:
        _compiled[key] = _build(variant, S, num_devices)
    return _compiled[key]


def _detect_variant(m: np.ndarray):
    """m: (S, S) fp32 additive mask. Returns (variant, maskT-or-None)."""
    S = m.shape[0]
    idx = np.arange(S)
    upper = idx[None, :] > idx[:, None]
    if (m[~upper] == 0).all() and (m[upper] < -1e30).all():
        return "causal", None
    if not m.any():
        return "dense_nomask", None
    # generic fallback: pre-scale by sqrt(HD) so exp(SCALE*(s+m')) == exp(s/sqrt(HD)+m)
    mt = np.maximum(m.T.astype(np.float64) * np.sqrt(HD), -3e38)
    return "dense_mask", np.ascontiguousarray(mt.astype(np.float32)).astype(BF16)


def kernel(**inputs) -> np.ndarray:
    global LAST_RESULT
    from concourse import bass_utils

    if os.environ.get("BASS_TRACE"):
        # no artifact share in this container; keep traces local
        bass_utils.upload_artifacts = lambda tmpdir: tmpdir

    hs = np.asarray(inputs["hidden_states"], dtype=np.float32)
    mask = np.asarray(inputs["attention_mask"], dtype=np.float32)
    Wq = np.asarray(inputs["Wq"], dtype=np.float32)
    Wk = np.asarray(inputs["Wk"], dtype=np.float32)
    Wv = np.asarray(inputs["Wv"], dtype=np.float32)
    Wo = np.asarray(inputs["Wo"], dtype=np.float32)

    B, S, H_ = hs.shape
    assert B == 1 and H_ == H
    x = hs.reshape(S, H_)
    variant, mt = _detect_variant(mask.reshape(S, S))
    nc = _get_compiled(variant, S)

    xt = np.ascontiguousarray(x.T).astype(BF16)
    in_maps = []
    for c in range(NCORES):
        kv = c // 2
        im = dict(
            xt=xt,
            wq=np.ascontiguousarray(Wq[:, 256 * c : 256 * (c + 1)]).astype(BF16),
            wk=np.ascontiguousarray(Wk[:, 128 * kv : 128 * (kv + 1)]).astype(BF16),
            wv=np.ascontiguousarray(Wv[:, 128 * kv : 128 * (kv + 1)]).astype(BF16),
            wo=np.ascontiguousarray(Wo[256 * c : 256 * (c + 1), :]).astype(BF16),
        )
        if mt is not None:
            im["maskt"] = mt
        in_maps.append(im)

    res = bass_utils.run_bass_kernel_spmd(nc, in_maps, core_ids=list(range(NCORES)))
    LAST_RESULT = res

    acc = res.results[0]["out"].astype(np.float64)
    for c in range(1, NCORES):
        acc += res.results[c]["out"].astype(np.float64)
    return acc.astype(np.float32).reshape(B, S, H_)
